# revision 36
# baseline (speedup 1.0000x reference)
"""MHA forward kernel for Trainium2 (Bass/Tile), sharded over (batch, head)
pairs across 8 NeuronCores.

Math (per (b,h) pair):
    out = softmax(Q K^T / sqrt(64) + bias) @ V     # bias broadcast over (b,h)

Device-side decomposition (everything transposed so the S x S score matrix
never needs an on-chip transpose). Q is host pre-scaled by 1/(8*32) so the
MM1 PSUM holds s/32:
    scoresT/32 = K^T Q''                             (PE)
    ACT path (k-tiles 0-13):  es = exp(32*s32)       (ACT activation, scale=32)
                              p  = es * ebiasT       (DVE 2x-mode multiply)
    DVE path (k-tiles 14-15): t = s32 + bias/32      (custom DVE op pair:
                              u = 1+t+c2 t^2+c3 t^3   cubic base, then u^32
                              p = u^32                via 5 squarings; keeps
                                                      ~1 of 8 exp chunks off
                                                      the saturated ACT)
    outT[d, q], sums[q] = [V | ones] matmul accum over k   (PE)
    host epilogue: out[q, d] = outT[d, q] / sums[q]

Schedule: qt-major over pairs so each exp(bias) q-chunk serves 4 pairs
(~60 GB/s DMA vs 240 pair-major). ACT paces at ~1.09us per 2-ktile chunk;
the DVE path drops ACT to 7 of 8 chunks. Final divide + transpose on host.
"""

import os
import sys

import numpy as np

for _p in ("/opt/trn_rl_repo",):
    if _p not in sys.path and os.path.isdir(_p):
        sys.path.insert(0, _p)

B, H, S, D = 2, 16, 2048, 64
N_CORES = 8
PAIRS = B * H                     # 32
PPC = PAIRS // N_CORES            # 4 pairs per core
SCALE = 1.0 / 8.0                 # 1/sqrt(64)
EXP_N = 32                        # exp(x) = base(x/EXP_N)^EXP_N on the DVE path
# Remez-fit cubic base coefficients for (((c3 t + c2) t + 1) t + 1)^32 ~ e^(32t)
C2_POLY = 0.5026260923
C3_POLY = 0.1666659222

KT = S // 128                     # 16 k-tiles of 128
QTILE = 512
QT = S // QTILE                   # 4 q-tiles
GROUP = 2                         # k-tiles per chunk (2 PSUM banks)
NG = KT // GROUP                  # 8 chunks per (pair, qt)
NDVE = int(os.environ.get("NDVE", "1"))   # trailing chunks on the DVE path
KT_ACT = KT - NDVE * GROUP        # leading k-tiles on the ACT path
SC_BUFS = int(os.environ.get("SC_BUFS", "3"))
ES_BUFS = int(os.environ.get("ES_BUFS", "4"))
P_BUFS = int(os.environ.get("P_BUFS", "7"))
U_BUFS = int(os.environ.get("U_BUFS", "3"))
LAG = int(os.environ.get("LAG", "6"))

_CACHE = {}


def _register_dve_ops():
    """Register the two custom DVE ops via the documented extension point
    (concourse.dve_ops.OPS). Idempotent."""
    from concourse import dve_ops
    from concourse.dve_spec import Spec, Src0, Src1, C0, C1, One, sq, lower
    from concourse.dve_spec import _has_src1 as has_src1

    if "ops" in _CACHE:
        return _CACHE["ops"]

    t = Src0 + Src1
    u = (C0 * C1) * t + C0          # c3 = C0*C1 (hoisted), c2 = C0
    u = u * t + One
    u = u * t + One
    spec_base = Spec(
        body=u,
        reference=lambda in0, in1, s0, s1, imm2: (
            ((s0 * s1) * (in0 + in1) + s0) * (in0 + in1) + 1.0
        )
        * (in0 + in1)
        + 1.0,
    )
    p = sq(sq(sq(sq(sq(Src0)))))
    spec_pow = Spec(body=p, reference=lambda in0, in1, s0, s1, imm2: in0**32)

    ops = []
    for name, spec in (("EXPB32_ANT", spec_base), ("POW32_ANT", spec_pow)):
        if name in dve_ops._SUB_OPCODE_FOR_NAME:
            ops.append(next(o for o in dve_ops.OPS if o.name == name))
            continue
        row = dve_ops._CUSTOM_DVE_ROW_BASE + len(dve_ops.OPS)
        assert row < 0x20
        shas = {}
        for ver in ("v3", "v4"):
            s = dve_ops.DveOpSpec(
                name=name,
                opcode=row,
                uops=lower(spec, ver=ver),
                rd1_en=has_src1(spec),
            )
            shas[ver] = s.sha(ver)
        op = dve_ops.DveOp(name, spec, subdim=False, uops_sha=shas)
        dve_ops.OPS.append(op)
        dve_ops.CUSTOM_DVE_SPECS[name] = spec
        dve_ops._SUB_OPCODE_FOR_NAME[name] = row
        ops.append(op)
    _CACHE["ops"] = tuple(ops)
    return _CACHE["ops"]


def _build_nc():
    import concourse.mybir as mybir
    import concourse.tile as tile
    from concourse import bacc

    exp_base_op, pow_op = _register_dve_ops()

    f32 = mybir.dt.float32
    f16 = mybir.dt.float16
    bf16 = mybir.dt.bfloat16
    nc = bacc.Bacc(None)

    # q duplicated into both partition halves; k packed even-ktiles into
    # partitions 0-63 and odd-ktiles into 64-127 -> row-tiled MM1 runs two
    # k-tiles concurrently in the two 64-row halves of the PE array.
    qT = nc.declare_dram_parameter("qT", [PPC, 128, S], bf16, isOutput=False)
    kT = nc.declare_dram_parameter("kT", [PPC, 128, KT // 2, 128], bf16, isOutput=False)
    # v1: [pair][p][kt][d] so each partition's line is 16*65*2B contiguous
    v1 = nc.declare_dram_parameter("v1", [PPC, 128, KT, D + 1], bf16, isOutput=False)
    # exp(bias)^T for the ACT path, [qt][p][kt][q], kt < KT_ACT
    ebT = nc.declare_dram_parameter("ebT", [QT, 128, KT_ACT, QTILE], bf16, isOutput=False)
    # bias^T/32 for the DVE path, [qt][p][kt][q], kt >= KT_ACT
    bpT = nc.declare_dram_parameter(
        "bpT", [QT, 128, KT - KT_ACT, QTILE], f16, isOutput=False
    )
    outT = nc.declare_dram_parameter("outT", [PPC, QT, D + 1, QTILE], f32, isOutput=True)

    with tile.TileContext(nc) as tc:
        with (
            tc.tile_pool(name="eb", bufs=1) as eb_pool,
            tc.tile_pool(name="qk", bufs=1) as qk_pool,
            tc.tile_pool(name="vv", bufs=1) as v_pool,
            tc.tile_pool(name="es", bufs=ES_BUFS) as es_pool,
            tc.tile_pool(name="uu", bufs=U_BUFS) as u_pool,
            tc.tile_pool(name="pp", bufs=P_BUFS) as p_pool,
            tc.tile_pool(name="ob", bufs=2) as ob_pool,
            tc.tile_pool(name="sc", bufs=SC_BUFS, space="PSUM") as sc_pool,
            tc.tile_pool(name="acc", bufs=2, space="PSUM") as acc_pool,
        ):
            eb_sb = eb_pool.tile([128, QT, KT_ACT, QTILE], bf16)
            bp_sb = eb_pool.tile([128, QT, KT - KT_ACT, QTILE], f16)

            # warm the ACT exp table (1.3us load) under the DMA head so the
            # first real exp doesn't pay it
            warm = eb_pool.tile([1, 1], f32, name="warm", tag="warm")
            nc.vector.memset(warm[:], 0.0)
            nc.scalar.activation(
                warm[:], warm[:], mybir.ActivationFunctionType.Exp,
                scale=float(EXP_N),
            )

            # Pair 0 chunked small so MM1 starts within ~1us of launch; the
            # ebT qt0 stream follows immediately (needed from the first mult).
            qk_tiles = {}
            for p in range(PPC):
                q_sb = qk_pool.tile([128, S], bf16, name="q_sb", tag=f"q{p}")
                k_sb = qk_pool.tile(
                    [128, KT // 2, 128], bf16, name="k_sb", tag=f"k{p}"
                )
                qk_tiles[p] = (q_sb, k_sb)
                if p == 0:
                    nc.sync.dma_start(k_sb[:], kT[p])
                    nc.sync.dma_start(q_sb[:, 0:QTILE], qT[p][:, 0:QTILE])

            # same transfer shapes as before, reordered by first-use time:
            # eb qt0 sub-chunk 0 unblocks the first multiply ~3us sooner;
            # pair-0's remaining q columns are only needed at the qt1 phase.
            v_tiles = {}
            v_tiles[0] = v_pool.tile([128, KT, D + 1], bf16, name="v_sb", tag="v0")
            q0_sb = qk_tiles[0][0]
            nc.sync.dma_start(eb_sb[:, 0, 0:4, :], ebT[0][:, 0:4, :])
            nc.sync.dma_start(v_tiles[0][:], v1[0])
            for kc in range(4, KT_ACT, 4):
                kc1 = min(kc + 4, KT_ACT)
                nc.sync.dma_start(eb_sb[:, 0, kc:kc1, :], ebT[0][:, kc:kc1, :])
            nc.sync.dma_start(q0_sb[:, QTILE:], qT[0][:, QTILE:])
            nc.sync.dma_start(bp_sb[:, 0], bpT[0])
            for p in range(1, PPC):
                q_sb, k_sb = qk_tiles[p]
                nc.sync.dma_start(q_sb[:], qT[p])
                nc.sync.dma_start(k_sb[:], kT[p])
                v_tiles[p] = v_pool.tile(
                    [128, KT, D + 1], bf16, name="v_sb", tag=f"v{p}"
                )
                nc.sync.dma_start(v_tiles[p][:], v1[p])
            for qc in range(1, QT):
                nc.sync.dma_start(eb_sb[:, qc], ebT[qc])
                nc.sync.dma_start(bp_sb[:, qc], bpT[qc])

            # ---- chunk stream: qt-major over pairs --------------------------
            stream = []  # (p, qt, g)
            for qt in range(QT):
                for p in range(PPC):
                    for g in range(NG):
                        stream.append((p, qt, g))

            state = {}

            def produce(p, qt, g):
                q_sb, k_sb = qk_tiles[p]
                qsl = slice(qt * QTILE, (qt + 1) * QTILE)
                s_psum = sc_pool.tile([128, GROUP, QTILE], f32, tag="sc")
                # row-tiled pair: even k-tile in array rows 0-63, odd in 64-127
                nc.tensor.matmul(
                    s_psum[:, 0, :],
                    k_sb[0:64, g, :],
                    q_sb[0:64, qsl],
                    start=True,
                    stop=True,
                    tile_position=(0, 0),
                )
                nc.tensor.matmul(
                    s_psum[:, 1, :],
                    k_sb[64:128, g, :],
                    q_sb[64:128, qsl],
                    start=True,
                    stop=True,
                    tile_position=(64, 0),
                )
                p_sb = p_pool.tile([128, GROUP, QTILE], bf16, tag="p")
                kt0 = g * GROUP
                if kt0 >= KT_ACT:
                    # DVE path: cubic base of exp(x/32), then ^32
                    u = u_pool.tile([128, GROUP, QTILE], mybir.dt.float32, tag="u")
                    nc.vector._custom_dve(
                        exp_base_op,
                        out=u[:],
                        in0=s_psum[:],
                        in1=bp_sb[:, qt, kt0 - KT_ACT : kt0 - KT_ACT + GROUP, :],
                        s0=C2_POLY,
                        s1=C3_POLY / C2_POLY,
                    )
                    nc.vector._custom_dve(pow_op, out=p_sb[:], in0=u[:])
                else:
                    es = es_pool.tile([128, GROUP, QTILE], bf16, tag="es")
                    nc.scalar.activation(
                        es[:], s_psum[:], mybir.ActivationFunctionType.Exp,
                        scale=float(EXP_N),
                    )
                    nc.vector.tensor_mul(
                        p_sb[:],
                        es[:],
                        eb_sb[:, qt, kt0 : kt0 + GROUP, :],
                    )
                return p_sb

            def consume(p, qt, g, p_sb):
                v_sb = v_tiles[p]
                st = state[(p, qt)]
                for j in range(GROUP):
                    kt = g * GROUP + j
                    nc.tensor.matmul(
                        st,
                        v_sb[:, kt, :],
                        p_sb[:, j, :],
                        start=(kt == 0),
                        stop=(kt == KT - 1),
                    )

            def epilogue(p, qt):
                o_psum = state.pop((p, qt))
                o_sb = ob_pool.tile([D + 1, QTILE], f32, tag="osb")
                nc.vector.tensor_scalar_mul(o_sb[:], o_psum[:], 1.0)
                nc.sync.dma_start(outT[p, qt], o_sb[:])

            pending = []  # (p, qt, g, p_sb)
            for i, (p, qt, g) in enumerate(stream):
                if (p, qt) not in state:
                    state[(p, qt)] = acc_pool.tile(
                        [D + 1, QTILE], mybir.dt.float32, name="osum", tag="osum"
                    )
                p_sb = produce(p, qt, g)
                pending.append((p, qt, g, p_sb))
                # steady-state lag LAG; tapered to 0 over the last produces so
                # the final MM2 flush overlaps the tail of the stream
                target = min(LAG, len(stream) - 1 - i)
                while len(pending) > target:
                    pp, pq, pg, ps = pending.pop(0)
                    consume(pp, pq, pg, ps)
                    if pg == NG - 1:
                        epilogue(pp, pq)

    return nc


def _get_nc():
    if "nc" not in _CACHE:
        nc = _build_nc()
        nc.finalize()
        _CACHE["nc"] = nc
    return _CACHE["nc"]


def _make_in_maps(mat1, mat2, mat3, bias):
    import ml_dtypes

    bf16 = ml_dtypes.bfloat16
    q = np.asarray(mat1, dtype=np.float32).reshape(PAIRS, S, D) * np.float32(
        SCALE / EXP_N
    )
    k = np.asarray(mat2, dtype=np.float32).reshape(PAIRS, S, D)
    # qT duplicated into both partition halves: [pair, 128, S]
    qT2 = np.concatenate([q.transpose(0, 2, 1)] * 2, axis=1)
    # kT packed [pair, 128, KT//2, 128]: partitions 0-63 = even k-tiles,
    # 64-127 = odd k-tiles (d-major within each half)
    kTr = k.transpose(0, 2, 1).reshape(PAIRS, D, KT // 2, 2, 128)
    kT2 = np.concatenate([kTr[:, :, :, 0, :], kTr[:, :, :, 1, :]], axis=1)
    v = np.asarray(mat3, dtype=np.float32).reshape(PAIRS, S, D)
    v1 = np.concatenate([v, np.ones((PAIRS, S, 1), np.float32)], axis=2)
    v1 = np.ascontiguousarray(
        v1.reshape(PAIRS, KT, 128, D + 1).transpose(0, 2, 1, 3).astype(bf16)
    )
    bT = np.asarray(bias, dtype=np.float32).reshape(S, S).T  # [k, q]
    bT4 = bT.reshape(KT, 128, QT, QTILE)
    # ACT path: exp(bias), [qt][p][kt][q] over k-tiles < KT_ACT
    ebT = np.ascontiguousarray(
        np.exp(bT4[:KT_ACT]).transpose(2, 1, 0, 3).astype(bf16)
    )
    # DVE path: bias/32 in f16, k-tiles >= KT_ACT
    bpT = np.ascontiguousarray(
        (bT4[KT_ACT:] / EXP_N).transpose(2, 1, 0, 3).astype(np.float16)
    )

    in_maps = []
    for c in range(N_CORES):
        sl = slice(c * PPC, (c + 1) * PPC)
        in_maps.append(
            {
                "qT": np.ascontiguousarray(qT2[sl].astype(bf16)),
                "kT": np.ascontiguousarray(kT2[sl].astype(bf16)),
                "v1": v1[sl],
                "ebT": ebT,
                "bpT": bpT,
            }
        )
    return in_maps


def kernel(mat1, mat2, mat3, bias):
    from concourse.bass_utils import run_bass_kernel_spmd

    in_maps = _make_in_maps(mat1, mat2, mat3, bias)
    nc = _get_nc()
    _CACHE["in_maps"] = in_maps
    res = run_bass_kernel_spmd(nc, in_maps, list(range(N_CORES)))
    outs = []
    for c in range(N_CORES):
        oT = res.results[c]["outT"]            # [PPC, QT, D+1, QTILE] f32
        oT = oT.transpose(0, 2, 1, 3).reshape(PPC, D + 1, S)
        o = oT[:, :D, :] / oT[:, D : D + 1, :]  # divide by softmax sums
        outs.append(o.transpose(0, 2, 1))       # [PPC, S, D]
    full = np.concatenate(outs, axis=0).reshape(B, H, S, D)
    return np.ascontiguousarray(full.astype(np.float32))


# revision 37
# speedup vs baseline: 1.1726x; 1.1726x over previous
"""MHA forward kernel for Trainium2 (Bass/Tile), sharded over (batch, head)
pairs across 8 NeuronCores.

Math (per (b,h) pair):
    out = softmax(Q K^T / sqrt(64) + bias) @ V     # bias broadcast over (b,h)

Device-side decomposition (everything transposed so the S x S score matrix
never needs an on-chip transpose). Q is host pre-scaled by 1/(8*32) so the
MM1 PSUM holds s/32:
    scoresT/32 = K^T Q''                             (PE)
    ACT path (k-tiles 0-13):  es = exp(32*s32)       (ACT activation, scale=32)
                              p  = es * ebiasT       (DVE 2x-mode multiply)
    DVE path (k-tiles 14-15): t = s32 + bias/32      (custom DVE op pair:
                              u = 1+t+c2 t^2+c3 t^3   cubic base, then u^32
                              p = u^32                via 5 squarings; keeps
                                                      ~1 of 8 exp chunks off
                                                      the saturated ACT)
    outT[d, q], sums[q] = [V | ones] matmul accum over k   (PE)
    host epilogue: out[q, d] = outT[d, q] / sums[q]

Schedule: qt-major over pairs so each exp(bias) q-chunk serves 4 pairs
(~60 GB/s DMA vs 240 pair-major). ACT paces at ~1.09us per 2-ktile chunk;
the DVE path drops ACT to 7 of 8 chunks. Final divide + transpose on host.
"""

import os
import sys

import numpy as np

for _p in ("/opt/trn_rl_repo",):
    if _p not in sys.path and os.path.isdir(_p):
        sys.path.insert(0, _p)

B, H, S, D = 2, 16, 2048, 64
N_CORES = 8
PAIRS = B * H                     # 32
PPC = PAIRS // N_CORES            # 4 pairs per core
SCALE = 1.0 / 8.0                 # 1/sqrt(64)
EXP_N = 32                        # exp(x) = base(x/EXP_N)^EXP_N on the DVE path
# Remez-fit cubic base coefficients for (((c3 t + c2) t + 1) t + 1)^32 ~ e^(32t)
C2_POLY = 0.5026260923
C3_POLY = 0.1666659222

KT = S // 128                     # 16 k-tiles of 128
QTILE = 512
QT = S // QTILE                   # 4 q-tiles
GROUP = 2                         # k-tiles per chunk (2 PSUM banks)
NG = KT // GROUP                  # 8 chunks per (pair, qt)
NDVE = int(os.environ.get("NDVE", "1"))   # trailing chunks on the DVE path
KT_ACT = KT - NDVE * GROUP        # leading k-tiles on the ACT path
SC_BUFS = int(os.environ.get("SC_BUFS", "3"))
ES_BUFS = int(os.environ.get("ES_BUFS", "4"))
P_BUFS = int(os.environ.get("P_BUFS", "7"))
U_BUFS = int(os.environ.get("U_BUFS", "3"))
LAG = int(os.environ.get("LAG", "6"))

_CACHE = {}


def _register_dve_ops():
    """Register the two custom DVE ops via the documented extension point
    (concourse.dve_ops.OPS). Idempotent."""
    from concourse import dve_ops
    from concourse.dve_spec import Spec, Src0, Src1, C0, C1, One, sq, lower
    from concourse.dve_spec import _has_src1 as has_src1

    if "ops" in _CACHE:
        return _CACHE["ops"]

    t = Src0 + Src1
    u = (C0 * C1) * t + C0          # c3 = C0*C1 (hoisted), c2 = C0
    u = u * t + One
    u = u * t + One
    spec_base = Spec(
        body=u,
        reference=lambda in0, in1, s0, s1, imm2: (
            ((s0 * s1) * (in0 + in1) + s0) * (in0 + in1) + 1.0
        )
        * (in0 + in1)
        + 1.0,
    )
    p = sq(sq(sq(sq(sq(Src0)))))
    spec_pow = Spec(body=p, reference=lambda in0, in1, s0, s1, imm2: in0**32)

    ops = []
    for name, spec in (("EXPB32_ANT", spec_base), ("POW32_ANT", spec_pow)):
        if name in dve_ops._SUB_OPCODE_FOR_NAME:
            ops.append(next(o for o in dve_ops.OPS if o.name == name))
            continue
        row = dve_ops._CUSTOM_DVE_ROW_BASE + len(dve_ops.OPS)
        assert row < 0x20
        shas = {}
        for ver in ("v3", "v4"):
            s = dve_ops.DveOpSpec(
                name=name,
                opcode=row,
                uops=lower(spec, ver=ver),
                rd1_en=has_src1(spec),
            )
            shas[ver] = s.sha(ver)
        op = dve_ops.DveOp(name, spec, subdim=False, uops_sha=shas)
        dve_ops.OPS.append(op)
        dve_ops.CUSTOM_DVE_SPECS[name] = spec
        dve_ops._SUB_OPCODE_FOR_NAME[name] = row
        ops.append(op)
    _CACHE["ops"] = tuple(ops)
    return _CACHE["ops"]


def _build_nc():
    import concourse.mybir as mybir
    import concourse.tile as tile
    from concourse import bacc

    exp_base_op, pow_op = _register_dve_ops()

    f32 = mybir.dt.float32
    f16 = mybir.dt.float16
    bf16 = mybir.dt.bfloat16
    nc = bacc.Bacc(None)

    # q duplicated into both partition halves; k packed even-ktiles into
    # partitions 0-63 and odd-ktiles into 64-127 -> row-tiled MM1 runs two
    # k-tiles concurrently in the two 64-row halves of the PE array.
    qT = nc.declare_dram_parameter("qT", [PPC, 128, S], bf16, isOutput=False)
    kT = nc.declare_dram_parameter("kT", [PPC, 128, KT // 2, 128], bf16, isOutput=False)
    # v1: [pair][p][kt][d] so each partition's line is 16*65*2B contiguous
    v1 = nc.declare_dram_parameter("v1", [PPC, 128, KT, D + 1], bf16, isOutput=False)
    # exp(bias)^T for the ACT path, [qt][p][kt][q], kt < KT_ACT
    ebT = nc.declare_dram_parameter("ebT", [QT, 128, KT_ACT, QTILE], bf16, isOutput=False)
    # bias^T/32 for the DVE path, [qt][p][kt][q], kt >= KT_ACT
    bpT = nc.declare_dram_parameter(
        "bpT", [QT, 128, KT - KT_ACT, QTILE], f16, isOutput=False
    )
    outT = nc.declare_dram_parameter("outT", [PPC, QT, D + 1, QTILE], f32, isOutput=True)

    with tile.TileContext(nc) as tc:
        with (
            tc.tile_pool(name="eb", bufs=1) as eb_pool,
            tc.tile_pool(name="qk", bufs=1) as qk_pool,
            tc.tile_pool(name="vv", bufs=1) as v_pool,
            tc.tile_pool(name="es", bufs=ES_BUFS) as es_pool,
            tc.tile_pool(name="uu", bufs=U_BUFS) as u_pool,
            tc.tile_pool(name="pp", bufs=P_BUFS) as p_pool,
            tc.tile_pool(name="ob", bufs=2) as ob_pool,
            tc.tile_pool(name="sc", bufs=SC_BUFS, space="PSUM") as sc_pool,
            tc.tile_pool(name="acc", bufs=2, space="PSUM") as acc_pool,
        ):
            eb_sb = eb_pool.tile([128, QT, KT_ACT, QTILE], bf16)
            bp_sb = eb_pool.tile([128, QT, KT - KT_ACT, QTILE], f16)

            # warm the ACT exp table (1.3us load) under the DMA head so the
            # first real exp doesn't pay it
            warm = eb_pool.tile([1, 1], f32, name="warm", tag="warm")
            nc.vector.memset(warm[:], 0.0)
            nc.scalar.activation(
                warm[:], warm[:], mybir.ActivationFunctionType.Exp,
                scale=float(EXP_N),
            )

            # Pair 0 chunked small so MM1 starts within ~1us of launch; the
            # ebT qt0 stream follows immediately (needed from the first mult).
            qk_tiles = {}
            for p in range(PPC):
                q_sb = qk_pool.tile([128, S], bf16, name="q_sb", tag=f"q{p}")
                k_sb = qk_pool.tile(
                    [128, KT // 2, 128], bf16, name="k_sb", tag=f"k{p}"
                )
                qk_tiles[p] = (q_sb, k_sb)
                if p == 0:
                    nc.sync.dma_start(k_sb[:], kT[p])
                    nc.sync.dma_start(q_sb[:, 0:QTILE], qT[p][:, 0:QTILE])
                    nc.sync.dma_start(q_sb[:, QTILE:], qT[p][:, QTILE:])

            v_tiles = {}
            v_tiles[0] = v_pool.tile([128, KT, D + 1], bf16, name="v_sb", tag="v0")
            nc.sync.dma_start(v_tiles[0][:], v1[0])
            # qt0 tables in sub-chunks so the first mults unblock quickly
            for kc in range(0, KT_ACT, 4):
                kc1 = min(kc + 4, KT_ACT)
                nc.sync.dma_start(eb_sb[:, 0, kc:kc1, :], ebT[0][:, kc:kc1, :])
            nc.sync.dma_start(bp_sb[:, 0], bpT[0])
            for p in range(1, PPC):
                q_sb, k_sb = qk_tiles[p]
                nc.sync.dma_start(q_sb[:], qT[p])
                nc.sync.dma_start(k_sb[:], kT[p])
                v_tiles[p] = v_pool.tile(
                    [128, KT, D + 1], bf16, name="v_sb", tag=f"v{p}"
                )
                nc.sync.dma_start(v_tiles[p][:], v1[p])
            for qc in range(1, QT):
                nc.sync.dma_start(eb_sb[:, qc], ebT[qc])
                nc.sync.dma_start(bp_sb[:, qc], bpT[qc])

            # ---- chunk stream: qt-major over pairs --------------------------
            stream = []  # (p, qt, g)
            for qt in range(QT):
                for p in range(PPC):
                    for g in range(NG):
                        stream.append((p, qt, g))

            state = {}

            def produce(p, qt, g):
                q_sb, k_sb = qk_tiles[p]
                qsl = slice(qt * QTILE, (qt + 1) * QTILE)
                s_psum = sc_pool.tile([128, GROUP, QTILE], f32, tag="sc")
                # row-tiled pair: even k-tile in array rows 0-63, odd in 64-127
                nc.tensor.matmul(
                    s_psum[:, 0, :],
                    k_sb[0:64, g, :],
                    q_sb[0:64, qsl],
                    start=True,
                    stop=True,
                    tile_position=(0, 0),
                )
                nc.tensor.matmul(
                    s_psum[:, 1, :],
                    k_sb[64:128, g, :],
                    q_sb[64:128, qsl],
                    start=True,
                    stop=True,
                    tile_position=(64, 0),
                )
                p_sb = p_pool.tile([128, GROUP, QTILE], bf16, tag="p")
                kt0 = g * GROUP
                if kt0 >= KT_ACT:
                    # DVE path: cubic base of exp(x/32), then ^32
                    u = u_pool.tile([128, GROUP, QTILE], mybir.dt.float32, tag="u")
                    nc.vector._custom_dve(
                        exp_base_op,
                        out=u[:],
                        in0=s_psum[:],
                        in1=bp_sb[:, qt, kt0 - KT_ACT : kt0 - KT_ACT + GROUP, :],
                        s0=C2_POLY,
                        s1=C3_POLY / C2_POLY,
                    )
                    nc.vector._custom_dve(pow_op, out=p_sb[:], in0=u[:])
                else:
                    es = es_pool.tile([128, GROUP, QTILE], bf16, tag="es")
                    nc.scalar.activation(
                        es[:], s_psum[:], mybir.ActivationFunctionType.Exp,
                        scale=float(EXP_N),
                    )
                    nc.vector.tensor_mul(
                        p_sb[:],
                        es[:],
                        eb_sb[:, qt, kt0 : kt0 + GROUP, :],
                    )
                return p_sb

            def consume(p, qt, g, p_sb):
                v_sb = v_tiles[p]
                st = state[(p, qt)]
                for j in range(GROUP):
                    kt = g * GROUP + j
                    nc.tensor.matmul(
                        st,
                        v_sb[:, kt, :],
                        p_sb[:, j, :],
                        start=(kt == 0),
                        stop=(kt == KT - 1),
                    )

            def epilogue(p, qt):
                o_psum = state.pop((p, qt))
                o_sb = ob_pool.tile([D + 1, QTILE], f32, tag="osb")
                nc.vector.tensor_scalar_mul(o_sb[:], o_psum[:], 1.0)
                nc.sync.dma_start(outT[p, qt], o_sb[:])

            pending = []  # (p, qt, g, p_sb)
            for i, (p, qt, g) in enumerate(stream):
                if (p, qt) not in state:
                    state[(p, qt)] = acc_pool.tile(
                        [D + 1, QTILE], mybir.dt.float32, name="osum", tag="osum"
                    )
                p_sb = produce(p, qt, g)
                pending.append((p, qt, g, p_sb))
                # steady-state lag LAG; tapered to 0 over the last produces so
                # the final MM2 flush overlaps the tail of the stream
                target = min(LAG, len(stream) - 1 - i)
                while len(pending) > target:
                    pp, pq, pg, ps = pending.pop(0)
                    consume(pp, pq, pg, ps)
                    if pg == NG - 1:
                        epilogue(pp, pq)

    return nc


def _get_nc():
    if "nc" not in _CACHE:
        nc = _build_nc()
        nc.finalize()
        _CACHE["nc"] = nc
    return _CACHE["nc"]


def _make_in_maps(mat1, mat2, mat3, bias):
    import ml_dtypes

    bf16 = ml_dtypes.bfloat16
    q = np.asarray(mat1, dtype=np.float32).reshape(PAIRS, S, D) * np.float32(
        SCALE / EXP_N
    )
    k = np.asarray(mat2, dtype=np.float32).reshape(PAIRS, S, D)
    # qT duplicated into both partition halves: [pair, 128, S]
    qT2 = np.concatenate([q.transpose(0, 2, 1)] * 2, axis=1)
    # kT packed [pair, 128, KT//2, 128]: partitions 0-63 = even k-tiles,
    # 64-127 = odd k-tiles (d-major within each half)
    kTr = k.transpose(0, 2, 1).reshape(PAIRS, D, KT // 2, 2, 128)
    kT2 = np.concatenate([kTr[:, :, :, 0, :], kTr[:, :, :, 1, :]], axis=1)
    v = np.asarray(mat3, dtype=np.float32).reshape(PAIRS, S, D)
    v1 = np.concatenate([v, np.ones((PAIRS, S, 1), np.float32)], axis=2)
    v1 = np.ascontiguousarray(
        v1.reshape(PAIRS, KT, 128, D + 1).transpose(0, 2, 1, 3).astype(bf16)
    )
    bT = np.asarray(bias, dtype=np.float32).reshape(S, S).T  # [k, q]
    bT4 = bT.reshape(KT, 128, QT, QTILE)
    # ACT path: exp(bias), [qt][p][kt][q] over k-tiles < KT_ACT
    ebT = np.ascontiguousarray(
        np.exp(bT4[:KT_ACT]).transpose(2, 1, 0, 3).astype(bf16)
    )
    # DVE path: bias/32 in f16, k-tiles >= KT_ACT
    bpT = np.ascontiguousarray(
        (bT4[KT_ACT:] / EXP_N).transpose(2, 1, 0, 3).astype(np.float16)
    )

    in_maps = []
    for c in range(N_CORES):
        sl = slice(c * PPC, (c + 1) * PPC)
        in_maps.append(
            {
                "qT": np.ascontiguousarray(qT2[sl].astype(bf16)),
                "kT": np.ascontiguousarray(kT2[sl].astype(bf16)),
                "v1": v1[sl],
                "ebT": ebT,
                "bpT": bpT,
            }
        )
    return in_maps


def kernel(mat1, mat2, mat3, bias):
    from concourse.bass_utils import run_bass_kernel_spmd

    in_maps = _make_in_maps(mat1, mat2, mat3, bias)
    nc = _get_nc()
    _CACHE["in_maps"] = in_maps
    res = run_bass_kernel_spmd(nc, in_maps, list(range(N_CORES)))
    outs = []
    for c in range(N_CORES):
        oT = res.results[c]["outT"]            # [PPC, QT, D+1, QTILE] f32
        oT = oT.transpose(0, 2, 1, 3).reshape(PPC, D + 1, S)
        o = oT[:, :D, :] / oT[:, D : D + 1, :]  # divide by softmax sums
        outs.append(o.transpose(0, 2, 1))       # [PPC, S, D]
    full = np.concatenate(outs, axis=0).reshape(B, H, S, D)
    return np.ascontiguousarray(full.astype(np.float32))
